# revision 31
# baseline (speedup 1.0000x reference)
"""Trainium2 Bass kernel for Bahdanau-style attention (nn_Attention).

Reference computation (B=128, S=1024, D=512):
    proj = tanh(concat(dec, enc) @ W1.T + b1)        # [B, S, D]
    scores = proj @ W2.T (+ b2, cancels in softmax)  # [B, S]
    alpha = softmax(scores, axis=1)
    context = einsum('bs,bsd->bd', alpha, enc)       # [B, D]

Strategy v2: pure data-parallel over batch (16 rows/core, 8 cores).

Per-core dataflow:
  - The dominant matmul (enc @ W1enc^T, 8.6 GFLOP/core) runs in fp8-e4m3
    DoubleRow mode: each matmul contracts a PAIR of 128-wide d-chunks
    (K=256) in 512 cycles -- 2x the bf16 rate.  W1's e4m3 quantization
    error is cancelled on the HOST with W2-aware error-feedback
    rounding: per d-column, each weight's rounding direction (floor vs
    ceil on the e4m3 grid) is chosen to drive the running sum of
    W2_h * E[tanh'_h] * eps_(h,d) toward zero -- the only path W1 error
    reaches the output is scores = W2 . tanh(hidden), so zeroing the
    W2-projection of the error removes most of its effect at zero
    device cost.
  - hiddenT layout [h', hc, s]: PSUM evacuated by ScalarE tanh with the
    per-(hc,b) bias pdb1 fused, scale=2^-14 undoing the fp8 scaling.
  - scores are computed in the "flipped" form: stationary = hT s-block
    [128h', 128s], moving = W2 chunk [128,1] -> out scoresT [128s', 1]
    columns, so the N=1 matmuls cost ~a weight load instead of a full
    512-cycle stream, and scores land with s on partitions.
  - exp on ScalarE (no max subtraction; scores are O(+-3) here), alpha
    kept UNNORMALIZED in bf16; the softmax denominator is summed on the
    host from the alpha tensor itself (exact consistency).
  - context also flipped: stationary = encN d-chunk [128s', 128d],
    moving = alpha column [128,1] -> ctxT [128d', b] accumulated over
    s-chunks.  Host divides by the denominator and reassembles.
Host side: shard batch, pack enc as (a) e4m3 d-major layout for the
proj matmul and (b) bf16 s-major layout for the context matmul.
"""

import numpy as np
import ml_dtypes

B, S, D = 128, 1024, 512
N_CORES = 8
B_LOC = B // N_CORES          # 16
DC = D // 128                 # 4 chunks of 128 along d (and h)
DCP = DC // 2                 # 2 DoubleRow chunk-pairs along d
SBLK = 512                    # s block for proj tiles
NSB = S // SBLK               # 2
NSC = S // 128                # 8 s-chunks of 128

_NPBF = ml_dtypes.bfloat16
_NPF8 = ml_dtypes.float8_e4m3   # TRN fp8e4: IEEE-style, max +-240
_CACHE: dict = {}

ENC_SCALE = 32.0
W1E_SCALE = 512.0
PH_DESCALE = 1.0 / (ENC_SCALE * W1E_SCALE)   # 2^-14, exact


def _build():
    from contextlib import ExitStack
    import concourse.bass as bass
    import concourse.tile as tile
    from concourse import bacc, mybir

    f32, bf16 = mybir.dt.float32, mybir.dt.bfloat16
    f8e4 = mybir.dt.float8e4
    AF = mybir.ActivationFunctionType
    DR = mybir.MatmulPerfMode.DoubleRow

    nc = bacc.Bacc("TRN2", target_bir_lowering=False, debug=False,
                   num_devices=N_CORES)

    encT8 = nc.dram_tensor("encT8", [B_LOC, 128, NSB, DCP, 2, SBLK], f8e4,
                           kind="ExternalInput").ap()
    encN16 = nc.dram_tensor("encN16", [B_LOC, 128, NSC, D], bf16,
                            kind="ExternalInput").ap()
    w1e8 = nc.dram_tensor("w1e8", [128, DC, DCP, 2, 128], f8e4,
                          kind="ExternalInput").ap()
    w1dT = nc.dram_tensor("w1dT", [DC, 128, DC, 128], bf16,
                          kind="ExternalInput").ap()
    decT = nc.dram_tensor("decT", [DC, 128, B_LOC], bf16,
                          kind="ExternalInput").ap()
    b1c = nc.dram_tensor("b1c", [DC, 128, 1], f32, kind="ExternalInput").ap()
    w2c = nc.dram_tensor("w2c", [128, DC, 1], bf16, kind="ExternalInput").ap()
    ctx_out = nc.dram_tensor("ctx_out", [128, DC, B_LOC], f32,
                             kind="ExternalOutput").ap()
    alpha_out = nc.dram_tensor("alpha_out", [128, B_LOC, NSC], bf16,
                               kind="ExternalOutput").ap()

    with tile.TileContext(nc) as tc, ExitStack() as ctx:
        singles = ctx.enter_context(tc.tile_pool(name="singles", bufs=1))
        encT_pool = ctx.enter_context(tc.tile_pool(name="encTp", bufs=3))
        encN_pool = ctx.enter_context(tc.tile_pool(name="encNp", bufs=4))
        hT_pool = ctx.enter_context(tc.tile_pool(name="hTp", bufs=3))
        ph_pool = ctx.enter_context(tc.tile_pool(name="php", bufs=3,
                                                 space="PSUM"))
        sc_ps_pool = ctx.enter_context(tc.tile_pool(name="scps", bufs=1,
                                                    space="PSUM"))

        # --- DMA order: enc data for b=0 first so the PE can start ASAP;
        # pd-path weights next (needed before the first tanh). ---
        encT_tiles = {}
        encN_tiles = {}

        def dma_enc(b, ring=None):
            # encT via the sync HWDGE ring; encN via the scalar HWDGE ring
            # mid-loop (parallel descriptor-gen) but via sync at startup so
            # it cannot steal SDMA bandwidth from b=0/1's encT.
            encT_t = encT_pool.tile([128, NSB, DCP, 2, SBLK], f8e4,
                                    tag="encT")
            nc.sync.dma_start(out=encT_t, in_=encT8[b])
            encN_t = encN_pool.tile([128, NSC, D], bf16, tag="encN")
            (ring or nc.scalar).dma_start(out=encN_t, in_=encN16[b])
            encT_tiles[b] = encT_t
            encN_tiles[b] = encN_t

        # order: b=0 proj inputs, pd-path weights, then b=0/1 encN behind
        # them all on the same ring.  w1e8 + encT(0)'s sb0 half gate the
        # first real matmul, so they go first and encT(0) ships as halves.
        w1e_sb = singles.tile([128, DC, DCP, 2, 128], f8e4)
        nc.sync.dma_start(out=w1e_sb, in_=w1e8)
        encT_0 = encT_pool.tile([128, NSB, DCP, 2, SBLK], f8e4, tag="encT")
        for sb in range(NSB):
            nc.sync.dma_start(out=encT_0[:, sb], in_=encT8[0, :, sb])
        encT_tiles[0] = encT_0

        # PE clock warm-up: the HAM gate runs the array at 1/4..1/2 speed
        # until ~3us of continuous busy.  Burn that ramp on throwaway
        # matmuls while b=0's enc is still in flight, so the real proj
        # starts at full clock.
        warm_sb = singles.tile([128, SBLK], bf16, name="warm")
        nc.vector.memset(warm_sb, 0.0)
        warm_ps = ph_pool.tile([128, NSB, SBLK], f32, tag="ph", name="warmps")
        for i in range(8):
            nc.tensor.matmul(warm_ps[:, 0, :], lhsT=warm_sb[:, 0:128],
                             rhs=warm_sb, start=(i == 0), stop=(i == 7))
        w1d_slabs = []
        for hc in range(DC):
            w1d_hc = singles.tile([128, DC, 128], bf16, name=f"w1d_hc{hc}")
            nc.sync.dma_start(out=w1d_hc, in_=w1dT[hc])
            w1d_slabs.append(w1d_hc)
        dec_sb = singles.tile([128, DC, B_LOC], bf16)
        nc.sync.dma_start(out=dec_sb, in_=decT.rearrange("dc p b -> p dc b"))
        b1_sb = singles.tile([128, DC, 1], f32)
        nc.sync.dma_start(out=b1_sb, in_=b1c.rearrange("dc p o -> p dc o"))
        w2_sb = singles.tile([128, DC, 1], bf16)
        nc.sync.dma_start(out=w2_sb, in_=w2c)
        # encT(1) ahead of the encN's: it gates iter-1's proj, while encN(b)
        # is only read by ctx(b) a full iteration later.
        encT_1 = encT_pool.tile([128, NSB, DCP, 2, SBLK], f8e4, tag="encT")
        nc.sync.dma_start(out=encT_1, in_=encT8[1])
        encT_tiles[1] = encT_1
        encN_0 = encN_pool.tile([128, NSC, D], bf16, tag="encN")
        nc.sync.dma_start(out=encN_0, in_=encN16[0])
        encN_tiles[0] = encN_0
        encN_1 = encN_pool.tile([128, NSC, D], bf16, tag="encN")
        nc.sync.dma_start(out=encN_1, in_=encN16[1])
        encN_tiles[1] = encN_1

        pdb1 = singles.tile([128, DC, B_LOC], f32)
        alphaT = singles.tile([128, B_LOC, NSC], bf16)
        ctx_sb = singles.tile([128, DC, B_LOC], f32)

        # PSUM accumulators for flipped scores / context
        scsT = sc_ps_pool.tile([128, B_LOC, NSC], f32, name="scsT")
        ctxT = sc_ps_pool.tile([128, DC, B_LOC], f32, name="ctxT")

        def emit_pd():
            # pd = W1dec @ dec (+b1), evacuated per-hc into pdb1.  Emitted
            # first: the PE warms up on these (and ACT takes its
            # activation-table load) while b=0's enc is still streaming in.
            pd_ps = ph_pool.tile([128, NSB, SBLK], f32, tag="ph",
                                 name="pd_ps")
            for hc in range(DC):
                o = pd_ps[:, 0, hc * B_LOC:(hc + 1) * B_LOC]
                for dc in range(DC):
                    nc.tensor.matmul(o, lhsT=w1d_slabs[hc][:, dc, :],
                                     rhs=dec_sb[:, dc, :],
                                     start=(dc == 0), stop=(dc == DC - 1))
                nc.scalar.activation(out=pdb1[:, hc, :], in_=o,
                                     func=AF.Identity, bias=b1_sb[:, hc, :],
                                     scale=1.0)

        def emit_scores(b, hT):
            # scores: stationary hT s-block, moving W2 chunk -> [128s', 1];
            # then exp straight out of the scores PSUM (scores are O(+-3)).
            for sc in range(NSC):
                o = scsT[:, b, sc:sc + 1]
                for hc in range(DC):
                    nc.tensor.matmul(o,
                                     lhsT=hT[:, hc, sc * 128:(sc + 1) * 128],
                                     rhs=w2_sb[:, hc, :],
                                     start=(hc == 0), stop=(hc == DC - 1))
            nc.scalar.activation(out=alphaT[:, b, :], in_=scsT[:, b, :],
                                 func=AF.Exp, bias=0.0, scale=1.0)

        def emit_ctx(b, encN_t):
            # context: stationary encN d-chunk, moving alpha col -> ctxT
            for dc in range(DC):
                o = ctxT[:, dc, b:b + 1]
                for sc in range(NSC):
                    nc.tensor.matmul(
                        o,
                        lhsT=encN_t[:, sc, dc * 128:(dc + 1) * 128],
                        rhs=alphaT[:, b, sc:sc + 1],
                        start=(sc == 0), stop=(sc == NSC - 1))

        def emit_out(sl):
            nc.vector.tensor_copy(out=ctx_sb[:, :, sl], in_=ctxT[:, :, sl])
            nc.sync.dma_start(out=ctx_out[:, :, sl], in_=ctx_sb[:, :, sl])
            nc.sync.dma_start(out=alpha_out[:, sl, :], in_=alphaT[:, sl, :])

        hT_tiles = {}
        for b in range(B_LOC):
            if b + 2 < B_LOC:
                dma_enc(b + 2)
            if b >= 1:
                # scores/exp for b-1 first: the exp lands on ACT before
                # tanh(b), so the ctx matmuls never stall behind tanh.
                emit_scores(b - 1, hT_tiles.pop(b - 1))
            hT = hT_pool.tile([128, DC, S], bf16, tag="hT")
            encT_t = encT_tiles.pop(b)
            for hc in range(DC):
                ph = ph_pool.tile([128, NSB, SBLK], f32, tag="ph")
                for sb in range(NSB):
                    for dcp in range(DCP):
                        nc.tensor.matmul(
                            ph[:, sb, :],
                            lhsT=w1e_sb[:, hc, dcp, :, :],
                            rhs=encT_t[:, sb, dcp, :, :],
                            start=(dcp == 0), stop=(dcp == DCP - 1),
                            perf_mode=DR)
                if b == 0 and hc == 0:
                    emit_pd()
                nc.scalar.activation(out=hT[:, hc, :], in_=ph,
                                     func=AF.Tanh,
                                     bias=pdb1[:, hc, b:b + 1],
                                     scale=PH_DESCALE)
            hT_tiles[b] = hT
            if b >= 1:
                emit_ctx(b - 1, encN_tiles.pop(b - 1))
            if b == 8:
                emit_out(slice(0, 8))
            elif b == 12:
                emit_out(slice(8, 12))
            elif b == B_LOC - 1:
                emit_out(slice(12, B_LOC - 1))
        emit_scores(B_LOC - 1, hT_tiles.pop(B_LOC - 1))
        emit_ctx(B_LOC - 1, encN_tiles.pop(B_LOC - 1))
        emit_out(slice(B_LOC - 1, B_LOC))

    nc.compile()
    return nc


def _get_nc():
    if "nc" not in _CACHE:
        _CACHE["nc"] = _build()
    return _CACHE["nc"]


def _q8(x):
    return np.clip(x, -240.0, 240.0).astype(_NPF8)


def _w1e_feedback_round(W1e, W1d, dec, W2):
    """Quantize 512*W1e to e4m3 choosing per-weight rounding direction so
    that, per d-column, sum_h W2_h * E[tanh'_h] * eps(h,d) ~ 0.  W1 error
    reaches the output only through scores = W2 . tanh(hidden), so zeroing
    its W2-projection cancels most of the end-to-end effect."""
    A = (W1E_SCALE * W1e).astype(np.float32)             # [h, d]
    Aq = _q8(A).astype(np.float32)                       # nearest rounding
    eps = Aq - A
    # the other fp8 neighbor of each value (bit-increment on sign-magnitude)
    f = np.clip(A, -240, 240).astype(_NPF8)
    bits = f.view(np.uint8)
    away = (bits + 1).astype(np.uint8)                   # magnitude + 1
    toward = np.where((bits & 0x7F) == 0, (bits ^ 0x80) + 1, bits - 1) \
        .astype(np.uint8)
    away_v = np.clip(away.view(_NPF8).astype(np.float32), -240, 240)
    toward_v = toward.view(_NPF8).astype(np.float32)
    second = np.where(np.abs(Aq) >= np.abs(A), toward_v, away_v)
    second = np.where(np.isfinite(second), second, Aq)
    eps2 = second - A

    # a_h = W2_h * E[tanh'] with pre-activation ~ N(pd[b,h], sig_h^2)
    sig = np.sqrt((W1e ** 2).sum(axis=1) + (W1d ** 2).sum(axis=1))
    pd_full = dec @ W1d.T                                # [B, h]
    xs = np.linspace(-4, 4, 33, dtype=np.float64)
    gw = np.exp(-xs ** 2 / 2)
    gw /= gw.sum()
    mu = pd_full[:, :, None] + sig[None, :, None] * xs[None, None, :]
    c_h = (gw[None, None, :] / np.cosh(mu) ** 2).sum(axis=2).mean(axis=0)
    a_h = W2[0].astype(np.float64) * c_h

    Wout = Aq.copy()
    R = np.zeros(A.shape[1], dtype=np.float64)
    for h in np.argsort(np.abs(a_h))[::-1]:              # big terms first
        e1 = eps[h] * a_h[h]
        e2 = eps2[h] * a_h[h]
        pick2 = np.abs(R + e2) < np.abs(R + e1)
        Wout[h] = np.where(pick2, second[h], Aq[h])
        R += np.where(pick2, e2, e1)
    return Wout.astype(_NPF8)                            # [h, d], scaled


def _prep_in_maps(inputs):
    dec = np.asarray(inputs["decoder_hidden"], dtype=np.float32)
    enc = np.asarray(inputs["encoder_outputs"], dtype=np.float32)
    W1 = np.asarray(inputs["W1"], dtype=np.float32)
    b1 = np.asarray(inputs["b1"], dtype=np.float32)
    W2 = np.asarray(inputs["W2"], dtype=np.float32)

    W1d, W1e = W1[:, :D], W1[:, D:]

    # [h, d] e4m3 -> [p, hc, dcp, i, m] with d = (2*dcp + i)*128 + p
    Wq = _w1e_feedback_round(W1e, W1d, dec.astype(np.float64), W2)
    w1e8 = np.ascontiguousarray(
        Wq.reshape(DC, 128, DCP, 2, 128).transpose(4, 0, 2, 3, 1))

    def _slab(wT):
        return np.ascontiguousarray(
            wT.reshape(DC, 128, DC, 128).transpose(2, 1, 0, 3)).astype(_NPBF)

    w1dT = _slab(W1d.T)
    b1c = np.ascontiguousarray(b1).reshape(DC, 128, 1).astype(np.float32)
    w2c = np.ascontiguousarray(
        W2[0].reshape(DC, 128).T).reshape(128, DC, 1).astype(_NPBF)

    in_maps = []
    for c in range(N_CORES):
        sl = slice(c * B_LOC, (c + 1) * B_LOC)
        enc_c = enc[sl]                                  # [16, 1024, 512]
        # encT8[b, p, sb, dcp, i, s] = q8(32*enc[b, sb*512+s, (2dcp+i)*128+p])
        encT_c = np.ascontiguousarray(
            _q8(ENC_SCALE * enc_c).reshape(B_LOC, NSB, SBLK, DCP, 2, 128)
            .transpose(0, 5, 1, 3, 4, 2))
        # encN16[b, p, sc, d] = bf16(enc[b, sc*128+p, d])
        encN_c = np.ascontiguousarray(
            enc_c.reshape(B_LOC, NSC, 128, D).transpose(0, 2, 1, 3)
        ).astype(_NPBF)
        decT_c = np.ascontiguousarray(dec[sl].T).reshape(DC, 128, B_LOC) \
            .astype(_NPBF)
        in_maps.append({
            "encT8": encT_c, "encN16": encN_c, "w1e8": w1e8, "w1dT": w1dT,
            "decT": decT_c, "b1c": b1c, "w2c": w2c,
        })
    return in_maps


def _run(inputs, trace=False, **kw):
    from concourse.bass_utils import run_bass_kernel_spmd
    nc = _get_nc()
    in_maps = _prep_in_maps(inputs)
    res = run_bass_kernel_spmd(nc, in_maps, core_ids=list(range(N_CORES)),
                               trace=trace, **kw)
    outs = []
    for i in range(N_CORES):
        ctxT = res.results[i]["ctx_out"].astype(np.float32)   # [128, DC, B]
        alpha = res.results[i]["alpha_out"].astype(np.float32)  # [128, B, NSC]
        den = alpha.sum(axis=(0, 2))                          # [B_LOC]
        ctx = ctxT.transpose(2, 1, 0).reshape(B_LOC, D)       # [b, dc*128+p]
        outs.append(ctx / den[:, None])
    full = np.concatenate(outs, axis=0).astype(np.float32)
    return full, res


def kernel(**inputs) -> np.ndarray:
    full, _ = _run(inputs, trace=False)
    return full


# revision 32
# speedup vs baseline: 1.0034x; 1.0034x over previous
"""Trainium2 Bass kernel for Bahdanau-style attention (nn_Attention).

Reference computation (B=128, S=1024, D=512):
    proj = tanh(concat(dec, enc) @ W1.T + b1)        # [B, S, D]
    scores = proj @ W2.T (+ b2, cancels in softmax)  # [B, S]
    alpha = softmax(scores, axis=1)
    context = einsum('bs,bsd->bd', alpha, enc)       # [B, D]

Strategy v2: pure data-parallel over batch (16 rows/core, 8 cores).

Per-core dataflow:
  - The dominant matmul (enc @ W1enc^T, 8.6 GFLOP/core) runs in fp8-e4m3
    DoubleRow mode: each matmul contracts a PAIR of 128-wide d-chunks
    (K=256) in 512 cycles -- 2x the bf16 rate.  W1's e4m3 quantization
    error is cancelled on the HOST with W2-aware error-feedback
    rounding: per d-column, each weight's rounding direction (floor vs
    ceil on the e4m3 grid) is chosen to drive the running sum of
    W2_h * E[tanh'_h] * eps_(h,d) toward zero -- the only path W1 error
    reaches the output is scores = W2 . tanh(hidden), so zeroing the
    W2-projection of the error removes most of its effect at zero
    device cost.
  - hiddenT layout [h', hc, s]: PSUM evacuated by ScalarE tanh with the
    per-(hc,b) bias pdb1 fused, scale=2^-14 undoing the fp8 scaling.
  - scores are computed in the "flipped" form: stationary = hT s-block
    [128h', 128s], moving = W2 chunk [128,1] -> out scoresT [128s', 1]
    columns, so the N=1 matmuls cost ~a weight load instead of a full
    512-cycle stream, and scores land with s on partitions.
  - exp on ScalarE (no max subtraction; scores are O(+-3) here), alpha
    kept UNNORMALIZED in bf16; the softmax denominator is summed on the
    host from the alpha tensor itself (exact consistency).
  - context also flipped: stationary = encN d-chunk [128s', 128d],
    moving = alpha column [128,1] -> ctxT [128d', b] accumulated over
    s-chunks.  Host divides by the denominator and reassembles.
Host side: shard batch, pack enc as (a) e4m3 d-major layout for the
proj matmul and (b) bf16 s-major layout for the context matmul.
"""

import numpy as np
import ml_dtypes

B, S, D = 128, 1024, 512
N_CORES = 8
B_LOC = B // N_CORES          # 16
DC = D // 128                 # 4 chunks of 128 along d (and h)
DCP = DC // 2                 # 2 DoubleRow chunk-pairs along d
SBLK = 512                    # s block for proj tiles
NSB = S // SBLK               # 2
NSC = S // 128                # 8 s-chunks of 128

_NPBF = ml_dtypes.bfloat16
_NPF8 = ml_dtypes.float8_e4m3   # TRN fp8e4: IEEE-style, max +-240
_CACHE: dict = {}

ENC_SCALE = 32.0
W1E_SCALE = 512.0
PH_DESCALE = 1.0 / (ENC_SCALE * W1E_SCALE)   # 2^-14, exact


def _build():
    from contextlib import ExitStack
    import concourse.bass as bass
    import concourse.tile as tile
    from concourse import bacc, mybir

    f32, bf16 = mybir.dt.float32, mybir.dt.bfloat16
    f8e4 = mybir.dt.float8e4
    AF = mybir.ActivationFunctionType
    DR = mybir.MatmulPerfMode.DoubleRow

    nc = bacc.Bacc("TRN2", target_bir_lowering=False, debug=False,
                   num_devices=N_CORES)

    encT8 = nc.dram_tensor("encT8", [B_LOC, 128, NSB, DCP, 2, SBLK], f8e4,
                           kind="ExternalInput").ap()
    encN16 = nc.dram_tensor("encN16", [B_LOC, 128, NSC, D], bf16,
                            kind="ExternalInput").ap()
    w1e8 = nc.dram_tensor("w1e8", [128, DC, DCP, 2, 128], f8e4,
                          kind="ExternalInput").ap()
    w1dT = nc.dram_tensor("w1dT", [DC, 128, DC, 128], bf16,
                          kind="ExternalInput").ap()
    decT = nc.dram_tensor("decT", [DC, 128, B_LOC], bf16,
                          kind="ExternalInput").ap()
    b1c = nc.dram_tensor("b1c", [DC, 128, 1], f32, kind="ExternalInput").ap()
    w2c = nc.dram_tensor("w2c", [128, DC, 1], bf16, kind="ExternalInput").ap()
    ctx_out = nc.dram_tensor("ctx_out", [128, DC, B_LOC], f32,
                             kind="ExternalOutput").ap()
    alpha_out = nc.dram_tensor("alpha_out", [128, B_LOC, NSC], bf16,
                               kind="ExternalOutput").ap()

    with tile.TileContext(nc) as tc, ExitStack() as ctx:
        singles = ctx.enter_context(tc.tile_pool(name="singles", bufs=1))
        encT_pool = ctx.enter_context(tc.tile_pool(name="encTp", bufs=3))
        encN_pool = ctx.enter_context(tc.tile_pool(name="encNp", bufs=4))
        hT_pool = ctx.enter_context(tc.tile_pool(name="hTp", bufs=3))
        ph_pool = ctx.enter_context(tc.tile_pool(name="php", bufs=3,
                                                 space="PSUM"))
        sc_ps_pool = ctx.enter_context(tc.tile_pool(name="scps", bufs=1,
                                                    space="PSUM"))

        # --- DMA order: enc data for b=0 first so the PE can start ASAP;
        # pd-path weights next (needed before the first tanh). ---
        encT_tiles = {}
        encN_tiles = {}

        def dma_enc(b, ring=None):
            # encT via the sync HWDGE ring; encN via the scalar HWDGE ring
            # mid-loop (parallel descriptor-gen) but via sync at startup so
            # it cannot steal SDMA bandwidth from b=0/1's encT.
            encT_t = encT_pool.tile([128, NSB, DCP, 2, SBLK], f8e4,
                                    tag="encT")
            nc.sync.dma_start(out=encT_t, in_=encT8[b])
            encN_t = encN_pool.tile([128, NSC, D], bf16, tag="encN")
            (ring or nc.scalar).dma_start(out=encN_t, in_=encN16[b])
            encT_tiles[b] = encT_t
            encN_tiles[b] = encN_t

        # order: b=0 proj inputs, pd-path weights, then b=0/1 encN behind
        # them all on the same ring.  w1e8 + encT(0)'s sb0 half gate the
        # first real matmul, so they go first and encT(0) ships as halves.
        w1e_sb = singles.tile([128, DC, DCP, 2, 128], f8e4)
        nc.sync.dma_start(out=w1e_sb, in_=w1e8)
        encT_0 = encT_pool.tile([128, NSB, DCP, 2, SBLK], f8e4, tag="encT")
        for sb in range(NSB):
            nc.sync.dma_start(out=encT_0[:, sb], in_=encT8[0, :, sb])
        encT_tiles[0] = encT_0

        # PE clock warm-up: the HAM gate runs the array at 1/4..1/2 speed
        # until ~3us of continuous busy.  Burn that ramp on throwaway
        # matmuls while b=0's enc is still in flight, so the real proj
        # starts at full clock.
        warm_sb = singles.tile([128, SBLK], bf16, name="warm")
        nc.vector.memset(warm_sb, 0.0)
        warm_ps = ph_pool.tile([128, NSB, SBLK], f32, tag="ph", name="warmps")
        for i in range(8):
            nc.tensor.matmul(warm_ps[:, 0, :], lhsT=warm_sb[:, 0:128],
                             rhs=warm_sb, start=(i == 0), stop=(i == 7))
        w1d_slabs = []
        for hc in range(DC):
            w1d_hc = singles.tile([128, DC, 128], bf16, name=f"w1d_hc{hc}")
            nc.sync.dma_start(out=w1d_hc, in_=w1dT[hc])
            w1d_slabs.append(w1d_hc)
        dec_sb = singles.tile([128, DC, B_LOC], bf16)
        nc.sync.dma_start(out=dec_sb, in_=decT.rearrange("dc p b -> p dc b"))
        b1_sb = singles.tile([128, DC, 1], f32)
        nc.sync.dma_start(out=b1_sb, in_=b1c.rearrange("dc p o -> p dc o"))
        w2_sb = singles.tile([128, DC, 1], bf16)
        nc.sync.dma_start(out=w2_sb, in_=w2c)
        encN_0 = encN_pool.tile([128, NSC, D], bf16, tag="encN")
        nc.sync.dma_start(out=encN_0, in_=encN16[0])
        encN_tiles[0] = encN_0
        dma_enc(1, ring=nc.sync)

        pdb1 = singles.tile([128, DC, B_LOC], f32)
        alphaT = singles.tile([128, B_LOC, NSC], bf16)
        ctx_sb = singles.tile([128, DC, B_LOC], f32)

        # PSUM accumulators for flipped scores / context
        scsT = sc_ps_pool.tile([128, B_LOC, NSC], f32, name="scsT")
        ctxT = sc_ps_pool.tile([128, DC, B_LOC], f32, name="ctxT")

        def emit_pd():
            # pd = W1dec @ dec (+b1), evacuated per-hc into pdb1.  Emitted
            # first: the PE warms up on these (and ACT takes its
            # activation-table load) while b=0's enc is still streaming in.
            pd_ps = ph_pool.tile([128, NSB, SBLK], f32, tag="ph",
                                 name="pd_ps")
            for hc in range(DC):
                o = pd_ps[:, 0, hc * B_LOC:(hc + 1) * B_LOC]
                for dc in range(DC):
                    nc.tensor.matmul(o, lhsT=w1d_slabs[hc][:, dc, :],
                                     rhs=dec_sb[:, dc, :],
                                     start=(dc == 0), stop=(dc == DC - 1))
                nc.scalar.activation(out=pdb1[:, hc, :], in_=o,
                                     func=AF.Identity, bias=b1_sb[:, hc, :],
                                     scale=1.0)

        def emit_scores(b, hT):
            # scores: stationary hT s-block, moving W2 chunk -> [128s', 1];
            # then exp straight out of the scores PSUM (scores are O(+-3)).
            for sc in range(NSC):
                o = scsT[:, b, sc:sc + 1]
                for hc in range(DC):
                    nc.tensor.matmul(o,
                                     lhsT=hT[:, hc, sc * 128:(sc + 1) * 128],
                                     rhs=w2_sb[:, hc, :],
                                     start=(hc == 0), stop=(hc == DC - 1))
            nc.scalar.activation(out=alphaT[:, b, :], in_=scsT[:, b, :],
                                 func=AF.Exp, bias=0.0, scale=1.0)

        def emit_ctx(b, encN_t):
            # context: stationary encN d-chunk, moving alpha col -> ctxT
            for dc in range(DC):
                o = ctxT[:, dc, b:b + 1]
                for sc in range(NSC):
                    nc.tensor.matmul(
                        o,
                        lhsT=encN_t[:, sc, dc * 128:(dc + 1) * 128],
                        rhs=alphaT[:, b, sc:sc + 1],
                        start=(sc == 0), stop=(sc == NSC - 1))

        def emit_out(sl):
            nc.vector.tensor_copy(out=ctx_sb[:, :, sl], in_=ctxT[:, :, sl])
            nc.sync.dma_start(out=ctx_out[:, :, sl], in_=ctx_sb[:, :, sl])
            nc.sync.dma_start(out=alpha_out[:, sl, :], in_=alphaT[:, sl, :])

        hT_tiles = {}
        for b in range(B_LOC):
            if b + 2 < B_LOC:
                dma_enc(b + 2)
            if b >= 1:
                # scores/exp for b-1 first: the exp lands on ACT before
                # tanh(b), so the ctx matmuls never stall behind tanh.
                emit_scores(b - 1, hT_tiles.pop(b - 1))
            hT = hT_pool.tile([128, DC, S], bf16, tag="hT")
            encT_t = encT_tiles.pop(b)
            for hc in range(DC):
                ph = ph_pool.tile([128, NSB, SBLK], f32, tag="ph")
                for sb in range(NSB):
                    for dcp in range(DCP):
                        nc.tensor.matmul(
                            ph[:, sb, :],
                            lhsT=w1e_sb[:, hc, dcp, :, :],
                            rhs=encT_t[:, sb, dcp, :, :],
                            start=(dcp == 0), stop=(dcp == DCP - 1),
                            perf_mode=DR)
                if b == 0 and hc == 0:
                    emit_pd()
                nc.scalar.activation(out=hT[:, hc, :], in_=ph,
                                     func=AF.Tanh,
                                     bias=pdb1[:, hc, b:b + 1],
                                     scale=PH_DESCALE)
            hT_tiles[b] = hT
            if b >= 1:
                emit_ctx(b - 1, encN_tiles.pop(b - 1))
            if b == 8:
                emit_out(slice(0, 8))
            elif b == 12:
                emit_out(slice(8, 12))
            elif b == B_LOC - 1:
                emit_out(slice(12, B_LOC - 1))
        emit_scores(B_LOC - 1, hT_tiles.pop(B_LOC - 1))
        emit_ctx(B_LOC - 1, encN_tiles.pop(B_LOC - 1))
        emit_out(slice(B_LOC - 1, B_LOC))

    nc.compile()
    return nc


def _get_nc():
    if "nc" not in _CACHE:
        _CACHE["nc"] = _build()
    return _CACHE["nc"]


def _q8(x):
    return np.clip(x, -240.0, 240.0).astype(_NPF8)


def _w1e_feedback_round(W1e, W1d, dec, W2):
    """Quantize 512*W1e to e4m3 choosing per-weight rounding direction so
    that, per d-column, sum_h W2_h * E[tanh'_h] * eps(h,d) ~ 0.  W1 error
    reaches the output only through scores = W2 . tanh(hidden), so zeroing
    its W2-projection cancels most of the end-to-end effect."""
    A = (W1E_SCALE * W1e).astype(np.float32)             # [h, d]
    Aq = _q8(A).astype(np.float32)                       # nearest rounding
    eps = Aq - A
    # the other fp8 neighbor of each value (bit-increment on sign-magnitude)
    f = np.clip(A, -240, 240).astype(_NPF8)
    bits = f.view(np.uint8)
    away = (bits + 1).astype(np.uint8)                   # magnitude + 1
    toward = np.where((bits & 0x7F) == 0, (bits ^ 0x80) + 1, bits - 1) \
        .astype(np.uint8)
    away_v = np.clip(away.view(_NPF8).astype(np.float32), -240, 240)
    toward_v = toward.view(_NPF8).astype(np.float32)
    second = np.where(np.abs(Aq) >= np.abs(A), toward_v, away_v)
    second = np.where(np.isfinite(second), second, Aq)
    eps2 = second - A

    # a_h = W2_h * E[tanh'] with pre-activation ~ N(pd[b,h], sig_h^2)
    sig = np.sqrt((W1e ** 2).sum(axis=1) + (W1d ** 2).sum(axis=1))
    pd_full = dec @ W1d.T                                # [B, h]
    xs = np.linspace(-4, 4, 33, dtype=np.float64)
    gw = np.exp(-xs ** 2 / 2)
    gw /= gw.sum()
    mu = pd_full[:, :, None] + sig[None, :, None] * xs[None, None, :]
    c_h = (gw[None, None, :] / np.cosh(mu) ** 2).sum(axis=2).mean(axis=0)
    a_h = W2[0].astype(np.float64) * c_h

    Wout = Aq.copy()
    R = np.zeros(A.shape[1], dtype=np.float64)
    for h in np.argsort(np.abs(a_h))[::-1]:              # big terms first
        e1 = eps[h] * a_h[h]
        e2 = eps2[h] * a_h[h]
        pick2 = np.abs(R + e2) < np.abs(R + e1)
        Wout[h] = np.where(pick2, second[h], Aq[h])
        R += np.where(pick2, e2, e1)
    return Wout.astype(_NPF8)                            # [h, d], scaled


def _prep_in_maps(inputs):
    dec = np.asarray(inputs["decoder_hidden"], dtype=np.float32)
    enc = np.asarray(inputs["encoder_outputs"], dtype=np.float32)
    W1 = np.asarray(inputs["W1"], dtype=np.float32)
    b1 = np.asarray(inputs["b1"], dtype=np.float32)
    W2 = np.asarray(inputs["W2"], dtype=np.float32)

    W1d, W1e = W1[:, :D], W1[:, D:]

    # [h, d] e4m3 -> [p, hc, dcp, i, m] with d = (2*dcp + i)*128 + p
    Wq = _w1e_feedback_round(W1e, W1d, dec.astype(np.float64), W2)
    w1e8 = np.ascontiguousarray(
        Wq.reshape(DC, 128, DCP, 2, 128).transpose(4, 0, 2, 3, 1))

    def _slab(wT):
        return np.ascontiguousarray(
            wT.reshape(DC, 128, DC, 128).transpose(2, 1, 0, 3)).astype(_NPBF)

    w1dT = _slab(W1d.T)
    b1c = np.ascontiguousarray(b1).reshape(DC, 128, 1).astype(np.float32)
    w2c = np.ascontiguousarray(
        W2[0].reshape(DC, 128).T).reshape(128, DC, 1).astype(_NPBF)

    in_maps = []
    for c in range(N_CORES):
        sl = slice(c * B_LOC, (c + 1) * B_LOC)
        enc_c = enc[sl]                                  # [16, 1024, 512]
        # encT8[b, p, sb, dcp, i, s] = q8(32*enc[b, sb*512+s, (2dcp+i)*128+p])
        encT_c = np.ascontiguousarray(
            _q8(ENC_SCALE * enc_c).reshape(B_LOC, NSB, SBLK, DCP, 2, 128)
            .transpose(0, 5, 1, 3, 4, 2))
        # encN16[b, p, sc, d] = bf16(enc[b, sc*128+p, d])
        encN_c = np.ascontiguousarray(
            enc_c.reshape(B_LOC, NSC, 128, D).transpose(0, 2, 1, 3)
        ).astype(_NPBF)
        decT_c = np.ascontiguousarray(dec[sl].T).reshape(DC, 128, B_LOC) \
            .astype(_NPBF)
        in_maps.append({
            "encT8": encT_c, "encN16": encN_c, "w1e8": w1e8, "w1dT": w1dT,
            "decT": decT_c, "b1c": b1c, "w2c": w2c,
        })
    return in_maps


def _run(inputs, trace=False, **kw):
    from concourse.bass_utils import run_bass_kernel_spmd
    nc = _get_nc()
    in_maps = _prep_in_maps(inputs)
    res = run_bass_kernel_spmd(nc, in_maps, core_ids=list(range(N_CORES)),
                               trace=trace, **kw)
    outs = []
    for i in range(N_CORES):
        ctxT = res.results[i]["ctx_out"].astype(np.float32)   # [128, DC, B]
        alpha = res.results[i]["alpha_out"].astype(np.float32)  # [128, B, NSC]
        den = alpha.sum(axis=(0, 2))                          # [B_LOC]
        ctx = ctxT.transpose(2, 1, 0).reshape(B_LOC, D)       # [b, dc*128+p]
        outs.append(ctx / den[:, None])
    full = np.concatenate(outs, axis=0).astype(np.float32)
    return full, res


def kernel(**inputs) -> np.ndarray:
    full, _ = _run(inputs, trace=False)
    return full
